# revision 9
# baseline (speedup 1.0000x reference)
"""DendriticMLP Trainium2 kernel (8-core data-parallel over batch).

Strategy:
- Shard batch (4096 -> 8 x 512); replicate all weights.
- Feature-major on-chip layout: activations live as hT[unit(partition), batch(free)],
  so block matmuls chain without transposes and BatchNorm stats/apply are
  per-partition operations.
- BatchNorm uses exact full-batch stats via a 16 KB AllReduce of per-unit
  (sum, sum_sq); the collective overlaps with the dendrite matmuls, which do
  not depend on it.
- Dendrite argmax-|.|-gather is computed without gathers: running elementwise
  max and min over the 16 per-segment matmul outputs, then
  sel = where(max >= -min, max, min).
- All matmuls run in float32r (~11-bit operand rounding, 1 cycle/row, ~4x
  faster than fp32) with fp32 PSUM accumulation.
"""
import os
import sys

sys.path.insert(0, "/opt/trn_rl_repo")

import numpy as np

import concourse.bass as bass
import concourse.mybir as mybir
import concourse.tile as tile
from concourse import bacc
from concourse.bass_utils import run_bass_kernel_spmd

B, D, H, S, OUT = 4096, 1024, 2048, 16, 1000
CORES = 8
BS = B // CORES            # 512 rows per core
OUTP = 1024                # classifier outputs padded to 8*128
KT_D = D // 128            # 8 k-tiles for 1024-dim contractions
KT_H = H // 128            # 16 k-tiles for 2048-dim contractions
UT_H = H // 128            # 16 unit tiles per hidden layer
UT_O = OUTP // 128         # 8 unit tiles for classifier
BN_EPS = 1e-5

F32 = mybir.dt.float32
F32R = mybir.dt.float32r
F16 = mybir.dt.float16
AX = mybir.AxisListType
ALU = mybir.AluOpType
ACTF = mybir.ActivationFunctionType

LAST_EXEC_NS = None
_CACHE = {}
DBG_LAYERS = int(os.environ.get("DBG_LAYERS", "3"))
DBG_NO_COLL = bool(int(os.environ.get("DBG_NO_COLL", "0")))
DBG_NO_DEND = bool(int(os.environ.get("DBG_NO_DEND", "0")))


def _build_nc():
    nc = bacc.Bacc("TRN2", target_bir_lowering=False, debug=False,
                   num_devices=CORES)

    xT = nc.dram_tensor("xT", [2 * KT_D, 128, BS], F32R, kind="ExternalInput").ap()
    wr0 = nc.dram_tensor("wr0", [UT_H, 128, KT_D, 128], F32R, kind="ExternalInput").ap()
    wr1 = nc.dram_tensor("wr1", [UT_H, 128, KT_H, 128], F32R, kind="ExternalInput").ap()
    wr2 = nc.dram_tensor("wr2", [UT_H, 128, KT_H, 128], F32R, kind="ExternalInput").ap()
    wcr = nc.dram_tensor("wcr", [UT_O, 128, KT_H, 128], F32R, kind="ExternalInput").ap()
    swr = [
        nc.dram_tensor(f"swr{i}", [UT_H, S, 128, KT_D, 128], F32R,
                       kind="ExternalInput").ap()
        for i in range(3)
    ]
    br = nc.dram_tensor("br", [128, 3 * UT_H], F32, kind="ExternalInput").ap()
    bcr = nc.dram_tensor("bcr", [128, UT_O], F32, kind="ExternalInput").ap()
    outT = nc.dram_tensor("outT", [UT_O, 128, BS], F32, kind="ExternalOutput").ap()

    wr = [wr0, wr1, wr2]

    with tile.TileContext(nc) as tc:
        with (
            tc.tile_pool(name="pers", bufs=1) as pers,
            tc.tile_pool(name="wblk", bufs=2) as wpool,
            tc.tile_pool(name="swp", bufs=4) as swpool,
            tc.tile_pool(name="work", bufs=2) as work,
            tc.tile_pool(name="ob", bufs=2) as opool,
            tc.tile_pool(name="pb", bufs=4, space="PSUM") as pb,
            tc.tile_pool(name="pd", bufs=4, space="PSUM") as pd,
            tc.tile_pool(name="dram", bufs=1, space="DRAM") as dram,
        ):
            # persistent tiles
            xin_tiles = [pers.tile([128, BS], F32R, tag=f"xin{k}", name=f"xin{k}")
                         for k in range(KT_D)]
            ctx_tiles = [pers.tile([128, BS], F32R, tag=f"ctx{k}", name=f"ctx{k}")
                         for k in range(KT_D)]
            h_tiles = [pers.tile([128, BS], F32R, tag=f"h{k}", name=f"h{k}")
                       for k in range(UT_H)]
            y_tiles = [pers.tile([128, BS], F32, tag=f"y{k}", name=f"y{k}")
                       for k in range(UT_H)]
            bias_sb = pers.tile([128, 3 * UT_H], F32, tag="bias_sb", name="bias_sb")
            bc_sb = pers.tile([128, UT_O], F32, tag="bc_sb", name="bc_sb")

            for k in range(KT_D):
                nc.sync.dma_start(xin_tiles[k][:], xT[k])
                nc.sync.dma_start(ctx_tiles[k][:], xT[KT_D + k])
            nc.sync.dma_start(bias_sb[:], br)
            nc.sync.dma_start(bc_sb[:], bcr)

            ctx_r = [t[:] for t in ctx_tiles]

            for layer in range(DBG_LAYERS):
                kt_in = KT_D if layer == 0 else KT_H
                in_tiles = xin_tiles if layer == 0 else h_tiles

                stats_loc = pers.tile([128, 2 * UT_H], F32, tag=f"stl{layer}",
                                      name=f"stl{layer}")
                stats_glob = pers.tile([128, 2 * UT_H], F32, tag=f"stg{layer}",
                                       name=f"stg{layer}")

                # ---- block matmuls + local BN stats ----
                for ut in range(UT_H):
                    wchunk = wpool.tile([128, kt_in * 128], F32R, tag="wblk",
                                        name=f"w{layer}_{ut}")
                    nc.sync.dma_start(
                        wchunk[:],
                        wr[layer][ut].rearrange("p a b -> p (a b)"),
                    )
                    ps = pb.tile([128, BS], F32, tag="yblk", name=f"yp{layer}_{ut}")
                    wcr_r = wchunk[:]
                    for kt in range(kt_in):
                        nc.tensor.matmul(
                            ps[:],
                            wcr_r[:, kt * 128:(kt + 1) * 128],
                            in_tiles[kt][:],
                            start=(kt == 0),
                            stop=(kt == kt_in - 1),
                        )
                    y = y_tiles[ut]
                    nc.scalar.activation(
                        y[:], ps[:], ACTF.Identity,
                        bias=bias_sb[:, layer * UT_H + ut:layer * UT_H + ut + 1],
                    )
                    nc.vector.tensor_reduce(
                        stats_loc[:, ut:ut + 1], y[:], axis=AX.X, op=ALU.add)
                    sq = work.tile([128, BS], F32, tag="sq", name=f"sq{layer}_{ut}")
                    nc.scalar.activation(
                        sq[:], y[:], ACTF.Square,
                        accum_out=stats_loc[:, UT_H + ut:UT_H + ut + 1],
                    )

                # ---- all-reduce BN stats (overlaps with dendrites below) ----
                bnc_in = dram.tile([128, 2 * UT_H], F32, tag=f"bin{layer}",
                                   name=f"bin{layer}")
                bnc_out = dram.tile([128, 2 * UT_H], F32, addr_space="Shared",
                                    tag=f"bout{layer}", name=f"bout{layer}")
                if DBG_NO_COLL:
                    nc.vector.tensor_scalar_mul(stats_glob[:], stats_loc[:],
                                                float(CORES))
                else:
                    nc.sync.dma_start(bnc_in[:], stats_loc[:])
                    nc.gpsimd.collective_compute(
                        "AllReduce", ALU.add,
                        ins=[bnc_in.opt()],
                        outs=[bnc_out.opt()],
                        replica_groups=[list(range(CORES))],
                    )
                    nc.sync.dma_start(stats_glob[:], bnc_out[:])

                # BN coefficients: scale = 1/sqrt(var+eps), nbias = -mean*scale
                mean = pers.tile([128, UT_H], F32, tag=f"mean{layer}",
                                 name=f"mean{layer}")
                var = pers.tile([128, UT_H], F32, tag=f"var{layer}",
                                name=f"var{layer}")
                scale = pers.tile([128, UT_H], F32, tag=f"scale{layer}",
                                  name=f"scale{layer}")
                nbias = pers.tile([128, UT_H], F32, tag=f"nbias{layer}",
                                  name=f"nbias{layer}")
                msq = pers.tile([128, UT_H], F32, tag=f"msq{layer}",
                                name=f"msq{layer}")
                nc.vector.tensor_scalar_mul(mean[:], stats_glob[:, 0:UT_H], 1.0 / B)
                nc.vector.tensor_scalar_mul(var[:], stats_glob[:, UT_H:2 * UT_H],
                                            1.0 / B)
                # var = E[y^2] - mean^2  (msq = -mean^2, then add)
                nc.vector.scalar_tensor_tensor(
                    out=msq[:], in0=mean[:], scalar=-1.0, in1=mean[:],
                    op0=ALU.mult, op1=ALU.mult,
                )
                nc.vector.tensor_tensor(var[:], var[:], msq[:], op=ALU.add)
                nc.vector.tensor_scalar_add(var[:], var[:], BN_EPS)
                nc.scalar.sqrt(scale[:], var[:])
                nc.vector.reciprocal(scale[:], scale[:])
                nc.vector.scalar_tensor_tensor(
                    out=nbias[:], in0=mean[:], scalar=-1.0, in1=scale[:],
                    op0=ALU.mult, op1=ALU.mult,
                )

                # ---- dendrites ----
                if DBG_NO_DEND:
                    for ut in range(UT_H):
                        nc.scalar.activation(
                            h_tiles[ut][:], y_tiles[ut][:], ACTF.Relu,
                            bias=nbias[:, ut:ut + 1], scale=scale[:, ut:ut + 1],
                        )
                    continue
                for ut in range(UT_H):
                    mx = work.tile([128, BS], F32, tag="mx", name=f"mx{layer}_{ut}")
                    mn = work.tile([128, BS], F32, tag="mn", name=f"mn{layer}_{ut}")
                    for s in range(S):
                        swc = swpool.tile([128, KT_D * 128], F32R, tag="sw",
                                          name=f"sw{layer}_{ut}_{s}")
                        nc.sync.dma_start(
                            swc[:],
                            swr[layer][ut, s].rearrange("p a b -> p (a b)"),
                        )
                        psd = pd.tile([128, BS], F32, tag="pd",
                                      name=f"pd{layer}_{ut}_{s}")
                        swc_r = swc[:]
                        for kt in range(KT_D):
                            nc.tensor.matmul(
                                psd[:],
                                swc_r[:, kt * 128:(kt + 1) * 128],
                                ctx_r[kt],
                                start=(kt == 0),
                                stop=(kt == KT_D - 1),
                            )
                        if s == 0:
                            nc.scalar.copy(mx[:], psd[:])
                            nc.vector.tensor_copy(mn[:], psd[:])
                        else:
                            nc.vector.tensor_tensor(mx[:], mx[:], psd[:], op=ALU.max)
                            nc.vector.tensor_tensor(mn[:], mn[:], psd[:], op=ALU.min)
                    # sel = where(mx >= -mn, mx, mn); overwrite mn with sel
                    negmn = work.tile([128, BS], F32, tag="negmn",
                                      name=f"ng{layer}_{ut}")
                    nc.scalar.mul(negmn[:], mn[:], -1.0)
                    mask = work.tile([128, BS], mybir.dt.uint8, tag="mask",
                                     name=f"mk{layer}_{ut}")
                    nc.vector.tensor_tensor(mask[:], mx[:], negmn[:], op=ALU.is_ge)
                    nc.vector.copy_predicated(mn[:], mask[:], mx[:])
                    g = work.tile([128, BS], F32, tag="g", name=f"g{layer}_{ut}")
                    nc.scalar.activation(g[:], mn[:], ACTF.Sigmoid)
                    # h = relu(y*scale + nbias) * g
                    nc.scalar.activation(
                        h_tiles[ut][:], y_tiles[ut][:], ACTF.Relu,
                        bias=nbias[:, ut:ut + 1], scale=scale[:, ut:ut + 1],
                    )
                    nc.vector.tensor_tensor(h_tiles[ut][:], h_tiles[ut][:], g[:],
                                            op=ALU.mult)

            # ---- classifier ----
            for ut in range(UT_O):
                wchunk = wpool.tile([128, KT_H * 128], F32R, tag="wblk",
                                    name=f"wc_{ut}")
                nc.sync.dma_start(wchunk[:], wcr[ut].rearrange("p a b -> p (a b)"))
                ps = pb.tile([128, BS], F32, tag="yblk", name=f"cp{ut}")
                wcr_r = wchunk[:]
                for kt in range(KT_H):
                    nc.tensor.matmul(
                        ps[:],
                        wcr_r[:, kt * 128:(kt + 1) * 128],
                        h_tiles[kt][:],
                        start=(kt == 0),
                        stop=(kt == KT_H - 1),
                    )
                osb = opool.tile([128, BS], F32, tag="osb", name=f"osb{ut}")
                nc.scalar.activation(osb[:], ps[:], ACTF.Identity,
                                     bias=bc_sb[:, ut:ut + 1])
                nc.sync.dma_start(outT[ut], osb[:])

    nc.compile()
    return nc


def _prep_host(x, w0, b0, sw0, w1, b1, sw1, w2, b2, sw2, wc, bc):
    f = np.float32
    h16 = np.float16

    def _w_reorder(w, kt):  # w [H_out, K] -> [16ut, 128ki, kt, 128u]
        wT = np.ascontiguousarray(w.astype(f).T)          # [K, H_out]
        K, HO = wT.shape
        return np.ascontiguousarray(
            wT.reshape(kt, 128, HO // 128, 128).transpose(2, 1, 0, 3))

    def _sw_reorder(sw):  # [H, S, D] -> [16ut, S, 128ki, 8kt, 128u]
        r = sw.astype(f).reshape(UT_H, 128, S, KT_D, 128)
        return np.ascontiguousarray(r.transpose(0, 2, 4, 3, 1))

    wc_pad = np.zeros((OUTP, H), f)
    wc_pad[:OUT] = wc.astype(f)
    bc_pad = np.zeros((OUTP,), f)
    bc_pad[:OUT] = bc.astype(f)

    common = {
        "wr0": _w_reorder(w0, KT_D),
        "wr1": _w_reorder(w1, KT_H),
        "wr2": _w_reorder(w2, KT_H),
        "wcr": _w_reorder(wc_pad, KT_H),
        "swr0": _sw_reorder(sw0),
        "swr1": _sw_reorder(sw1),
        "swr2": _sw_reorder(sw2),
        "br": np.ascontiguousarray(
            np.stack([b0, b1, b2]).astype(f).reshape(3 * UT_H, 128).T),
        "bcr": np.ascontiguousarray(bc_pad.reshape(UT_O, 128).T),
    }
    in_maps = []
    for c in range(CORES):
        xs = np.ascontiguousarray(x[c * BS:(c + 1) * BS].astype(f).T)
        m = dict(common)
        m["xT"] = xs.reshape(2 * KT_D, 128, BS)
        in_maps.append(m)
    return in_maps


def kernel(**inputs):
    global LAST_EXEC_NS
    if "nc" not in _CACHE:
        _CACHE["nc"] = _build_nc()
    nc = _CACHE["nc"]

    in_maps = _prep_host(**inputs)

    trace = bool(int(os.environ.get("KERNEL_TRACE", "0")))
    if trace:
        try:
            sys.path.insert(0, "/root/problem/work")
            import ntff_shim
            ntff_shim.install()
        except Exception:
            trace = False

    res = run_bass_kernel_spmd(nc, in_maps, core_ids=list(range(CORES)),
                               trace=trace)
    LAST_EXEC_NS = res.exec_time_ns

    out = np.empty((B, OUT), np.float32)
    for c in range(CORES):
        oT = res.results[c]["outT"].reshape(OUTP, BS)
        out[c * BS:(c + 1) * BS] = oT[:OUT].T
    return out


# revision 10
# speedup vs baseline: 1.0732x; 1.0732x over previous
"""DendriticMLP Trainium2 kernel (8-core data-parallel over batch).

Strategy:
- Shard batch (4096 -> 8 x 512); replicate all weights.
- Feature-major on-chip layout: activations live as hT[unit(partition), batch(free)],
  so block matmuls chain without transposes and BatchNorm stats/apply are
  per-partition operations.
- BatchNorm uses exact full-batch stats via a 16 KB AllReduce of per-unit
  (sum, sum_sq); the collective overlaps with the dendrite matmuls, which do
  not depend on it.
- Dendrite argmax-|.|-gather is computed without gathers: running elementwise
  max and min over the 16 per-segment matmul outputs, then
  sel = where(max >= -min, max, min).
- All matmuls run in float32r (~11-bit operand rounding, 1 cycle/row, ~4x
  faster than fp32) with fp32 PSUM accumulation.
"""
import os
import sys

sys.path.insert(0, "/opt/trn_rl_repo")

import numpy as np

import concourse.bass as bass
import concourse.mybir as mybir
import concourse.tile as tile
from concourse import bacc
from concourse.bass_utils import run_bass_kernel_spmd

B, D, H, S, OUT = 4096, 1024, 2048, 16, 1000
CORES = 8
BS = B // CORES            # 512 rows per core
OUTP = 1024                # classifier outputs padded to 8*128
KT_D = D // 128            # 8 k-tiles for 1024-dim contractions
KT_H = H // 128            # 16 k-tiles for 2048-dim contractions
UT_H = H // 128            # 16 unit tiles per hidden layer
UT_O = OUTP // 128         # 8 unit tiles for classifier
BN_EPS = 1e-5

F32 = mybir.dt.float32
F32R = mybir.dt.float32r
F16 = mybir.dt.float16
AX = mybir.AxisListType
ALU = mybir.AluOpType
ACTF = mybir.ActivationFunctionType

LAST_EXEC_NS = None
_CACHE = {}
DBG_LAYERS = int(os.environ.get("DBG_LAYERS", "3"))
DBG_NO_COLL = bool(int(os.environ.get("DBG_NO_COLL", "0")))
DBG_NO_DEND = bool(int(os.environ.get("DBG_NO_DEND", "0")))


def _build_nc():
    nc = bacc.Bacc("TRN2", target_bir_lowering=False, debug=False,
                   num_devices=CORES)

    xT = nc.dram_tensor("xT", [2 * KT_D, 128, BS], F32R, kind="ExternalInput").ap()
    wr0 = nc.dram_tensor("wr0", [UT_H, 128, KT_D, 128], F32R, kind="ExternalInput").ap()
    wr1 = nc.dram_tensor("wr1", [UT_H, 128, KT_H, 128], F32R, kind="ExternalInput").ap()
    wr2 = nc.dram_tensor("wr2", [UT_H, 128, KT_H, 128], F32R, kind="ExternalInput").ap()
    wcr = nc.dram_tensor("wcr", [UT_O, 128, KT_H, 128], F32R, kind="ExternalInput").ap()
    swr = [
        nc.dram_tensor(f"swr{i}", [UT_H, S, 128, KT_D, 128], F32R,
                       kind="ExternalInput").ap()
        for i in range(3)
    ]
    br = nc.dram_tensor("br", [128, 3 * UT_H], F32, kind="ExternalInput").ap()
    bcr = nc.dram_tensor("bcr", [128, UT_O], F32, kind="ExternalInput").ap()
    outT = nc.dram_tensor("outT", [UT_O, 128, BS], F32, kind="ExternalOutput").ap()

    wr = [wr0, wr1, wr2]

    with tile.TileContext(nc) as tc:
        with (
            tc.tile_pool(name="pers", bufs=1) as pers,
            tc.tile_pool(name="wblk", bufs=2) as wpool,
            tc.tile_pool(name="swp", bufs=4) as swpool,
            tc.tile_pool(name="work", bufs=2) as work,
            tc.tile_pool(name="ob", bufs=2) as opool,
            tc.tile_pool(name="pb", bufs=2, space="PSUM") as pb,
            tc.tile_pool(name="pd", bufs=6, space="PSUM") as pd,
            tc.tile_pool(name="dram", bufs=1, space="DRAM") as dram,
        ):
            # persistent tiles
            xin_tiles = [pers.tile([128, BS], F32R, tag=f"xin{k}", name=f"xin{k}")
                         for k in range(KT_D)]
            ctx_tiles = [pers.tile([128, BS], F32R, tag=f"ctx{k}", name=f"ctx{k}")
                         for k in range(KT_D)]
            h_tiles = [pers.tile([128, BS], F32R, tag=f"h{k}", name=f"h{k}")
                       for k in range(UT_H)]
            y_tiles = [pers.tile([128, BS], F32, tag=f"y{k}", name=f"y{k}")
                       for k in range(UT_H)]
            bias_sb = pers.tile([128, 3 * UT_H], F32, tag="bias_sb", name="bias_sb")
            bc_sb = pers.tile([128, UT_O], F32, tag="bc_sb", name="bc_sb")

            for k in range(KT_D):
                nc.sync.dma_start(xin_tiles[k][:], xT[k])
                nc.sync.dma_start(ctx_tiles[k][:], xT[KT_D + k])
            nc.sync.dma_start(bias_sb[:], br)
            nc.sync.dma_start(bc_sb[:], bcr)

            ctx_r = [t[:] for t in ctx_tiles]

            for layer in range(DBG_LAYERS):
                kt_in = KT_D if layer == 0 else KT_H
                in_tiles = xin_tiles if layer == 0 else h_tiles

                stats_loc = pers.tile([128, 2 * UT_H], F32, tag=f"stl{layer}",
                                      name=f"stl{layer}")
                stats_glob = pers.tile([128, 2 * UT_H], F32, tag=f"stg{layer}",
                                       name=f"stg{layer}")

                # ---- block matmuls + local BN stats ----
                for ut in range(UT_H):
                    wchunk = wpool.tile([128, kt_in * 128], F32R, tag="wblk",
                                        name=f"w{layer}_{ut}")
                    nc.sync.dma_start(
                        wchunk[:],
                        wr[layer][ut].rearrange("p a b -> p (a b)"),
                    )
                    ps = pb.tile([128, BS], F32, tag="yblk", name=f"yp{layer}_{ut}")
                    wcr_r = wchunk[:]
                    for kt in range(kt_in):
                        nc.tensor.matmul(
                            ps[:],
                            wcr_r[:, kt * 128:(kt + 1) * 128],
                            in_tiles[kt][:],
                            start=(kt == 0),
                            stop=(kt == kt_in - 1),
                        )
                    y = y_tiles[ut]
                    nc.scalar.activation(
                        y[:], ps[:], ACTF.Identity,
                        bias=bias_sb[:, layer * UT_H + ut:layer * UT_H + ut + 1],
                    )
                    nc.vector.tensor_reduce(
                        stats_loc[:, ut:ut + 1], y[:], axis=AX.X, op=ALU.add)
                    sq = work.tile([128, BS], F32, tag="sq", name=f"sq{layer}_{ut}")
                    nc.scalar.activation(
                        sq[:], y[:], ACTF.Square,
                        accum_out=stats_loc[:, UT_H + ut:UT_H + ut + 1],
                    )

                # ---- all-reduce BN stats (overlaps with dendrites below) ----
                bnc_in = dram.tile([128, 2 * UT_H], F32, tag=f"bin{layer}",
                                   name=f"bin{layer}")
                bnc_out = dram.tile([128, 2 * UT_H], F32, addr_space="Shared",
                                    tag=f"bout{layer}", name=f"bout{layer}")
                if DBG_NO_COLL:
                    nc.vector.tensor_scalar_mul(stats_glob[:], stats_loc[:],
                                                float(CORES))
                else:
                    nc.sync.dma_start(bnc_in[:], stats_loc[:])
                    nc.gpsimd.collective_compute(
                        "AllReduce", ALU.add,
                        ins=[bnc_in.opt()],
                        outs=[bnc_out.opt()],
                        replica_groups=[list(range(CORES))],
                    )
                    nc.sync.dma_start(stats_glob[:], bnc_out[:])

                # BN coefficients: scale = 1/sqrt(var+eps), nbias = -mean*scale
                mean = pers.tile([128, UT_H], F32, tag=f"mean{layer}",
                                 name=f"mean{layer}")
                var = pers.tile([128, UT_H], F32, tag=f"var{layer}",
                                name=f"var{layer}")
                scale = pers.tile([128, UT_H], F32, tag=f"scale{layer}",
                                  name=f"scale{layer}")
                nbias = pers.tile([128, UT_H], F32, tag=f"nbias{layer}",
                                  name=f"nbias{layer}")
                msq = pers.tile([128, UT_H], F32, tag=f"msq{layer}",
                                name=f"msq{layer}")
                nc.vector.tensor_scalar_mul(mean[:], stats_glob[:, 0:UT_H], 1.0 / B)
                nc.vector.tensor_scalar_mul(var[:], stats_glob[:, UT_H:2 * UT_H],
                                            1.0 / B)
                # var = E[y^2] - mean^2  (msq = -mean^2, then add)
                nc.vector.scalar_tensor_tensor(
                    out=msq[:], in0=mean[:], scalar=-1.0, in1=mean[:],
                    op0=ALU.mult, op1=ALU.mult,
                )
                nc.vector.tensor_tensor(var[:], var[:], msq[:], op=ALU.add)
                nc.vector.tensor_scalar_add(var[:], var[:], BN_EPS)
                nc.scalar.sqrt(scale[:], var[:])
                nc.vector.reciprocal(scale[:], scale[:])
                nc.vector.scalar_tensor_tensor(
                    out=nbias[:], in0=mean[:], scalar=-1.0, in1=scale[:],
                    op0=ALU.mult, op1=ALU.mult,
                )

                # ---- dendrites ----
                if DBG_NO_DEND:
                    for ut in range(UT_H):
                        nc.scalar.activation(
                            h_tiles[ut][:], y_tiles[ut][:], ACTF.Relu,
                            bias=nbias[:, ut:ut + 1], scale=scale[:, ut:ut + 1],
                        )
                    continue
                for ut in range(UT_H):
                    mx = work.tile([128, BS], F32, tag="mx", name=f"mx{layer}_{ut}")
                    mn = work.tile([128, BS], F32, tag="mn", name=f"mn{layer}_{ut}")
                    for s in range(S):
                        swc = swpool.tile([128, KT_D * 128], F32R, tag="sw",
                                          name=f"sw{layer}_{ut}_{s}")
                        nc.sync.dma_start(
                            swc[:],
                            swr[layer][ut, s].rearrange("p a b -> p (a b)"),
                        )
                        psd = pd.tile([128, BS], F32, tag="pd",
                                      name=f"pd{layer}_{ut}_{s}")
                        swc_r = swc[:]
                        for kt in range(KT_D):
                            nc.tensor.matmul(
                                psd[:],
                                swc_r[:, kt * 128:(kt + 1) * 128],
                                ctx_r[kt],
                                start=(kt == 0),
                                stop=(kt == KT_D - 1),
                            )
                        if s == 0:
                            nc.scalar.copy(mx[:], psd[:])
                            nc.vector.tensor_copy(mn[:], psd[:])
                        else:
                            nc.vector.tensor_tensor(mx[:], mx[:], psd[:], op=ALU.max)
                            nc.vector.tensor_tensor(mn[:], mn[:], psd[:], op=ALU.min)
                    # sel = where(mx >= -mn, mx, mn); overwrite mn with sel
                    negmn = work.tile([128, BS], F32, tag="negmn",
                                      name=f"ng{layer}_{ut}")
                    nc.scalar.mul(negmn[:], mn[:], -1.0)
                    mask = work.tile([128, BS], mybir.dt.uint8, tag="mask",
                                     name=f"mk{layer}_{ut}")
                    nc.vector.tensor_tensor(mask[:], mx[:], negmn[:], op=ALU.is_ge)
                    nc.vector.copy_predicated(mn[:], mask[:], mx[:])
                    g = work.tile([128, BS], F32, tag="g", name=f"g{layer}_{ut}")
                    nc.scalar.activation(g[:], mn[:], ACTF.Sigmoid)
                    # h = relu(y*scale + nbias) * g
                    nc.scalar.activation(
                        h_tiles[ut][:], y_tiles[ut][:], ACTF.Relu,
                        bias=nbias[:, ut:ut + 1], scale=scale[:, ut:ut + 1],
                    )
                    nc.vector.tensor_tensor(h_tiles[ut][:], h_tiles[ut][:], g[:],
                                            op=ALU.mult)

            # ---- classifier ----
            for ut in range(UT_O):
                wchunk = wpool.tile([128, KT_H * 128], F32R, tag="wblk",
                                    name=f"wc_{ut}")
                nc.sync.dma_start(wchunk[:], wcr[ut].rearrange("p a b -> p (a b)"))
                ps = pb.tile([128, BS], F32, tag="yblk", name=f"cp{ut}")
                wcr_r = wchunk[:]
                for kt in range(KT_H):
                    nc.tensor.matmul(
                        ps[:],
                        wcr_r[:, kt * 128:(kt + 1) * 128],
                        h_tiles[kt][:],
                        start=(kt == 0),
                        stop=(kt == KT_H - 1),
                    )
                osb = opool.tile([128, BS], F32, tag="osb", name=f"osb{ut}")
                nc.scalar.activation(osb[:], ps[:], ACTF.Identity,
                                     bias=bc_sb[:, ut:ut + 1])
                nc.sync.dma_start(outT[ut], osb[:])

    nc.compile()
    return nc


def _prep_host(x, w0, b0, sw0, w1, b1, sw1, w2, b2, sw2, wc, bc):
    f = np.float32
    h16 = np.float16

    def _w_reorder(w, kt):  # w [H_out, K] -> [16ut, 128ki, kt, 128u]
        wT = np.ascontiguousarray(w.astype(f).T)          # [K, H_out]
        K, HO = wT.shape
        return np.ascontiguousarray(
            wT.reshape(kt, 128, HO // 128, 128).transpose(2, 1, 0, 3))

    def _sw_reorder(sw):  # [H, S, D] -> [16ut, S, 128ki, 8kt, 128u]
        r = sw.astype(f).reshape(UT_H, 128, S, KT_D, 128)
        return np.ascontiguousarray(r.transpose(0, 2, 4, 3, 1))

    wc_pad = np.zeros((OUTP, H), f)
    wc_pad[:OUT] = wc.astype(f)
    bc_pad = np.zeros((OUTP,), f)
    bc_pad[:OUT] = bc.astype(f)

    common = {
        "wr0": _w_reorder(w0, KT_D),
        "wr1": _w_reorder(w1, KT_H),
        "wr2": _w_reorder(w2, KT_H),
        "wcr": _w_reorder(wc_pad, KT_H),
        "swr0": _sw_reorder(sw0),
        "swr1": _sw_reorder(sw1),
        "swr2": _sw_reorder(sw2),
        "br": np.ascontiguousarray(
            np.stack([b0, b1, b2]).astype(f).reshape(3 * UT_H, 128).T),
        "bcr": np.ascontiguousarray(bc_pad.reshape(UT_O, 128).T),
    }
    in_maps = []
    for c in range(CORES):
        xs = np.ascontiguousarray(x[c * BS:(c + 1) * BS].astype(f).T)
        m = dict(common)
        m["xT"] = xs.reshape(2 * KT_D, 128, BS)
        in_maps.append(m)
    return in_maps


def kernel(**inputs):
    global LAST_EXEC_NS
    if "nc" not in _CACHE:
        _CACHE["nc"] = _build_nc()
    nc = _CACHE["nc"]

    in_maps = _prep_host(**inputs)

    trace = bool(int(os.environ.get("KERNEL_TRACE", "0")))
    if trace:
        try:
            sys.path.insert(0, "/root/problem/work")
            import ntff_shim
            ntff_shim.install()
        except Exception:
            trace = False

    res = run_bass_kernel_spmd(nc, in_maps, core_ids=list(range(CORES)),
                               trace=trace)
    LAST_EXEC_NS = res.exec_time_ns

    out = np.empty((B, OUT), np.float32)
    for c in range(CORES):
        oT = res.results[c]["outT"].reshape(OUTP, BS)
        out[c * BS:(c + 1) * BS] = oT[:OUT].T
    return out
